# revision 1
# baseline (speedup 1.0000x reference)
"""GNN message-passing (gather + segment-sum) on 8 TRN2 NeuronCores.

Strategy (edge-parallel, destination-sharded, no collectives):
  - Host: assign each destination node to one of 8*50 (core, block) buckets
    with a degree-balanced greedy packer (<=128 nodes per bucket, lo/hi edge
    sums equalized) so per-bucket edge counts pad tightly to 128-edge tiles.
    Each core's edges are grouped by destination bucket and split by source
    half (src < 25000 vs >= 25000, to fit int16 gather indices).
    x is converted to a bf16 hi/lo split table x2[r] = [bf16(x[r]),
    bf16(x[r] - hi)] (128 bf16 = 256 B per row, error ~2^-18 relative).
  - Device (per core, SPMD, fully independent):
      for each destination block:
        dma_gather   msgs[128e, T, 128] = x2[src]        (2 calls: lo/hi half)
        DVE is_equal sel[128e, T, 128] = (dst_rel == iota)     (bf16)
        PE matmul    psum[128, 128n] += msgs[:,t,:].T @ sel[:,t,:]  (T accums)
                     (psum rows 0:64 = hi sums, 64:128 = lo sums)
        DVE          outT[64, block*128:...] = psum[0:64] + psum[64:128]
      one DMA of outT [64, 6400] to DRAM.
  - Host: inverse-permute per-core outputs back to node order.

Collision-free by construction (no scatter-add): duplicate destinations are
combined by the one-hot matmul accumulation in PSUM. The gather is the
bottleneck (SWDGE descriptor rate, ~9-11 ns/row/queue across 4 queues); the
DVE one-hot build and PE matmuls overlap under it.
"""

import numpy as np
import ml_dtypes

import concourse.tile as tile
from concourse import bacc, mybir
from concourse import bass_utils

N_NODES = 50000
D = 64
N_CORES = 8
NODES_PER_CORE = N_NODES // N_CORES  # 6250
BLOCK = 128
SRC_SPLIT = 25000
PAD_REL = 999.0  # dst_rel value for padding edges (matches no iota column)


def assign_nodes(deg_lo, deg_hi, n_cores, n_blocks, block):
    """Greedy 2D balanced assignment of nodes to (core, block) buckets.

    Nodes (sorted by degree, descending) go to the bucket with minimal
    max(lo_sum, hi_sum) that still has node capacity. This equalizes both
    the lo and hi edge counts across all buckets, minimizing tile padding.

    Returns (core_of, block_of, pos_of): per-node arrays.
    """
    n_nodes = len(deg_lo)
    nb = n_cores * n_blocks
    lo_s = np.zeros(nb)
    hi_s = np.zeros(nb)
    cnt = np.zeros(nb, np.int64)
    assign = np.empty(n_nodes, np.int64)
    order = np.argsort(-(deg_lo + deg_hi), kind="stable")
    full_penalty = np.zeros(nb)
    for n in order:
        score = np.maximum(lo_s + deg_lo[n], hi_s + deg_hi[n]) + full_penalty
        b = int(np.argmin(score))
        assign[n] = b
        lo_s[b] += deg_lo[n]
        hi_s[b] += deg_hi[n]
        cnt[b] += 1
        if cnt[b] >= block:
            full_penalty[b] = 1e18
    core_of = assign // n_blocks
    block_of = assign % n_blocks
    # position within bucket: stable order of assignment
    pos_of = np.empty(n_nodes, np.int64)
    fill = np.zeros(nb, np.int64)
    for n in order:
        b = assign[n]
        pos_of[n] = fill[b]
        fill[b] += 1
    return core_of, block_of, pos_of


def bin_edges(edge_index, n_cores, nodes_per_core, block, src_split, n_blocks=None):
    """Bin edges into (core, dst-block, src-half) buckets with balanced
    node-to-bucket assignment, pad to common per-(block,half) tile counts.

    Returns:
      T_lo, T_hi: [n_blocks] int arrays, tiles (128 edges) per bucket
      src16: [n_cores, 128, tot_cols] int16 gather indices (wrapped+replicated)
      dstrel: [n_cores, 128, tot_tiles] bfloat16 relative dst (position-major)
      node_loc: (core_of, block_of, pos_of) for output reassembly
    """
    dst = np.asarray(edge_index[0], dtype=np.int64)
    src = np.asarray(edge_index[1], dtype=np.int64)
    n_nodes = n_cores * nodes_per_core
    if n_blocks is None:
        n_blocks = -(-nodes_per_core // block) + 1  # one extra for packing slack

    half = (src >= src_split).astype(np.int64)
    deg_lo = np.bincount(dst[half == 0], minlength=n_nodes)
    deg_hi = np.bincount(dst[half == 1], minlength=n_nodes)
    core_of, block_of, pos_of = assign_nodes(
        deg_lo, deg_hi, n_cores, n_blocks, block
    )

    core = core_of[dst]
    blk = block_of[dst]
    rel = pos_of[dst].astype(np.float32)
    bucket = (core * n_blocks + blk) * 2 + half

    order = np.argsort(bucket, kind="stable")
    src_s = src[order]
    rel_s = rel[order]

    counts = np.bincount(bucket, minlength=n_cores * n_blocks * 2).reshape(
        n_cores, n_blocks, 2
    )
    # common (across cores) tile count per (block, half); at least 1
    T = np.maximum(1, -(-counts.max(axis=0) // 128))  # [n_blocks, 2]
    T_lo, T_hi = T[:, 0], T[:, 1]
    tiles_per_block = T_lo + T_hi
    tot_tiles = int(tiles_per_block.sum())
    tot_edges = tot_tiles * 128

    # bucket start offsets (in padded edge positions), same for every core
    pad_sizes = T.reshape(-1) * 128  # [n_blocks*2] block-major, half-minor
    pad_starts = np.zeros(n_blocks * 2, dtype=np.int64)
    pad_starts[1:] = np.cumsum(pad_sizes)[:-1]

    src_pad = np.zeros((n_cores, tot_edges), dtype=np.int16)
    rel_pad = np.full((n_cores, tot_edges), PAD_REL, dtype=np.float32)

    cum = counts.reshape(n_cores, -1).cumsum(axis=1)
    starts_real = np.zeros((n_cores, n_blocks * 2), dtype=np.int64)
    starts_real[:, 1:] = cum[:, :-1]
    core_base = np.zeros(n_cores, dtype=np.int64)
    core_counts = counts.sum(axis=(1, 2))
    core_base[1:] = np.cumsum(core_counts)[:-1]

    for c in range(n_cores):
        cnts = counts[c].reshape(-1)
        for bh in range(n_blocks * 2):
            n = int(cnts[bh])
            if n == 0:
                continue
            s = int(core_base[c] + starts_real[c, bh])
            p = int(pad_starts[bh])
            sv = src_s[s : s + n]
            if bh % 2 == 1:
                sv = sv - src_split
            src_pad[c, p : p + n] = sv.astype(np.int16)
            rel_pad[c, p : p + n] = rel_s[s : s + n]

    # gather indices: wrapped in 16 partitions (idx i -> [i%16, i//16]),
    # replicated to all 8 gpsimd partition groups
    w = src_pad.reshape(n_cores, -1, 16).transpose(0, 2, 1)
    src16 = np.tile(w, (1, 8, 1)).copy()  # [n_cores, 128, cols]

    # dst_rel: edge position-major: position i -> [i%128, i//128]
    dstrel = (
        rel_pad.reshape(n_cores, -1, 128)
        .transpose(0, 2, 1)
        .astype(ml_dtypes.bfloat16)
        .copy()
    )  # [n_cores, 128, tot_tiles]

    return (
        T_lo.astype(int),
        T_hi.astype(int),
        src16,
        dstrel,
        (core_of, block_of, pos_of),
    )


def make_hilo_table(x):
    """x [N, 64] f32 -> [N, 128] bf16: cols 0:64 = bf16(x), 64:128 =
    bf16(x - hi). hi + lo reconstructs x to ~2^-18 relative."""
    hi = x.astype(ml_dtypes.bfloat16)
    lo = (x - hi.astype(np.float32)).astype(ml_dtypes.bfloat16)
    return np.concatenate([hi, lo], axis=1)


def build_program(T_lo, T_hi, n_rows, src_split, d=D, block=BLOCK, repeat=1,
                  msgs_bufs=6, sel_bufs=6, psum_bufs=8, single_packet=False):
    """Build the SPMD Bass program for given per-block tile counts.

    repeat > 1 wraps the block loop in a hardware For_i loop running the
    identical computation `repeat` times (device-time measurement by
    wall-clock slope; results unchanged — iterations overwrite outputs).
    """
    n_blocks = len(T_lo)
    out_cols = n_blocks * block
    tot_tiles = int((T_lo + T_hi).sum())
    d2 = 2 * d  # hi|lo row width in bf16 elements

    nc = bacc.Bacc(
        "TRN2",
        target_bir_lowering=False,
        debug=False,
        num_devices=N_CORES,
        num_swdge_queues=4,
    )
    x2 = nc.dram_tensor("x2", [n_rows, d2], mybir.dt.bfloat16, kind="ExternalInput")
    src16 = nc.dram_tensor(
        "src16", [128, tot_tiles * 8], mybir.dt.int16, kind="ExternalInput"
    )
    dstrel = nc.dram_tensor(
        "dstrel", [128, tot_tiles], mybir.dt.bfloat16, kind="ExternalInput"
    )
    iota_in = nc.dram_tensor(
        "iota", [128, block], mybir.dt.bfloat16, kind="ExternalInput"
    )
    out = nc.dram_tensor("out", [d, out_cols], mybir.dt.float32, kind="ExternalOutput")

    x_lo = x2.ap()[0:src_split, :]
    x_hi = x2.ap()[src_split:n_rows, :]

    with tile.TileContext(nc) as tc:
        with (
            tc.tile_pool(name="meta", bufs=1) as meta_pool,
            tc.tile_pool(name="msgs", bufs=msgs_bufs) as msgs_pool,
            tc.tile_pool(name="sel", bufs=sel_bufs) as sel_pool,
            tc.tile_pool(name="obuf", bufs=1) as obuf_pool,
            tc.tile_pool(name="psum", bufs=psum_bufs, space="PSUM") as psum_pool,
        ):
            src_t = meta_pool.tile([128, tot_tiles * 8], mybir.dt.int16)
            nc.sync.dma_start(src_t[:], src16.ap())
            rel_t = meta_pool.tile([128, tot_tiles], mybir.dt.bfloat16)
            nc.sync.dma_start(rel_t[:], dstrel.ap())
            iota_t = meta_pool.tile([128, block], mybir.dt.bfloat16)
            nc.sync.dma_start(iota_t[:], iota_in.ap())

            outbuf = obuf_pool.tile([d, out_cols], mybir.dt.float32)

            # Cap per-chunk tiles so SBUF pool size is input-independent
            # (skewed degree distributions can make single buckets huge).
            CHUNK_T = 16

            def body():
                off = 0  # tile offset of current bucket
                q = 0
                for b in range(n_blocks):
                    tl, th = int(T_lo[b]), int(T_hi[b])
                    tb = tl + th
                    # (source-table AP, first tile, tile count) per src half,
                    # split into chunks of at most CHUNK_T tiles
                    segs = []
                    for tab, t0, tn in ((x_lo, 0, tl), (x_hi, tl, th)):
                        for cs in range(0, tn, CHUNK_T):
                            segs.append((tab, t0 + cs, min(CHUNK_T, tn - cs)))

                    psum = psum_pool.tile([d2, block], mybir.dt.float32, space="PSUM")
                    done = 0
                    for tab, t0, tn in segs:
                        msgs = msgs_pool.tile(
                            [128, CHUNK_T, d2], mybir.dt.bfloat16, tag="msgs"
                        )
                        nc.gpsimd.dma_gather(
                            msgs[:, 0:tn, :],
                            tab,
                            src_t[:, (off + t0) * 8 : (off + t0 + tn) * 8],
                            tn * 128,
                            tn * 128,
                            d2,
                            queue_num=q % 4,
                            single_packet=single_packet and tn <= 8,
                        )
                        q += 1

                        sel = sel_pool.tile(
                            [128, CHUNK_T, block], mybir.dt.bfloat16, tag="sel"
                        )
                        nc.vector.tensor_tensor(
                            out=sel[:, 0:tn, :],
                            in0=rel_t[
                                :, off + t0 : off + t0 + tn
                            ].to_broadcast([128, tn, block]),
                            in1=iota_t[:]
                            .rearrange("p (o n) -> p o n", o=1)
                            .to_broadcast([128, tn, block]),
                            op=mybir.AluOpType.is_equal,
                        )

                        for t in range(tn):
                            nc.tensor.matmul(
                                out=psum[:],
                                lhsT=msgs[:, t, :],
                                rhs=sel[:, t, :],
                                start=(done + t == 0),
                                stop=(done + t == tb - 1),
                            )
                        done += tn
                    # psum rows 0:d = hi sums, d:2d = lo sums; combine.
                    # Engines may read only one PSUM input per op; the copy
                    # runs on the otherwise-idle Scalar engine to keep the
                    # DVE free for sel builds.
                    oslice = outbuf[:, b * block : (b + 1) * block]
                    nc.scalar.mul(oslice, psum[0:d, :], 1.0)
                    nc.vector.tensor_tensor(
                        out=oslice,
                        in0=oslice,
                        in1=psum[d : 2 * d, :],
                        op=mybir.AluOpType.add,
                    )
                    off += tb
                nc.sync.dma_start(out.ap(), outbuf[:])

            if repeat > 1:
                with tc.For_i(0, repeat, 1):
                    body()
            else:
                body()

    nc.compile()
    return nc


def make_iota():
    return np.broadcast_to(
        np.arange(BLOCK, dtype=np.float32)[None, :], (128, BLOCK)
    ).astype(ml_dtypes.bfloat16).copy()


def unshard_output(results, node_loc, block=BLOCK, n_nodes=N_NODES, d=D):
    core_of, block_of, pos_of = node_loc
    cols = block_of * block + pos_of
    out = np.empty((n_nodes, d), dtype=np.float32)
    for c in range(len(results)):
        mask = core_of == c
        out[mask] = results[c]["out"].T[cols[mask]]
    return out


def kernel(edge_index, x):
    edge_index = np.asarray(edge_index)
    x = np.ascontiguousarray(np.asarray(x, dtype=np.float32))
    T_lo, T_hi, src16, dstrel, node_loc = bin_edges(
        edge_index, N_CORES, NODES_PER_CORE, BLOCK, SRC_SPLIT
    )
    nc = build_program(T_lo, T_hi, N_NODES, SRC_SPLIT)

    x2 = make_hilo_table(x)
    iota = make_iota()
    in_maps = [
        {"x2": x2, "src16": src16[c], "dstrel": dstrel[c], "iota": iota}
        for c in range(N_CORES)
    ]
    res = bass_utils.run_bass_kernel_spmd(nc, in_maps, core_ids=list(range(N_CORES)))
    return unshard_output(res.results, node_loc)

